# revision 23
# baseline (speedup 1.0000x reference)
"""Multi-head self-attention TRN2 kernel, 8-way head-parallel, software-pipelined.

Reference computation (fp32):
    Q = x @ Wq.T; K = x @ Wk.T; V = x @ Wv.T        (split into 16 heads of 64)
    out = softmax(Q K^T / 8) V   per head, concat -> @ Wo.T

Sharding: 2 heads per core (e-block of 128 embed dims). Each core computes
its heads' attention output and a partial out-projection
    out_c = A_c @ Wo[:, e_c].T      (full shape, summed on host)

All matmuls fp16 (1 cyc/row on the PE); PSUM accumulation fp32.

The PE only reaches its 2.4GHz p-state after ~3us of continuous execution and
the in-order engine queue stalls on any not-yet-satisfied dependency, so the
kernel is built as one long software-pipelined instruction stream that keeps
the PE queue saturated:

  - scores run one kt-group ahead of the A.T@V accumulation: PE order is
    S(g+1), <fillers>, U(g), so the ACT exp(g) (the only cross-engine dep)
    completes while S(g+1) streams. Score PSUM tiles are double-buffered
    ([128,1024] x2 = 4 banks).
  - everything that is not scores/U (Q/K/V projections for the NEXT batch,
    the out-projection + softmax normalize of the PREVIOUS query block) is
    queued as filler closures popped between S and U, filling the PE slack
    that ACT's exp pace would otherwise leave.
  - PSUM po-tag tiles (proj, oproj, transpose, norm broadcast) open and meet
    their reader inside a single closure so the 2-buffer rotation can never
    interleave a long-lived accumulation with another alloc (deadlock).

Per-core dataflow (per batch):
  - QT/KT projections produce (128 = 2x64 head dims, T) with the embed
    contraction on partitions (x fed pre-transposed from host)
  - V.T produced the same way, then PE-transposed to (token, dv) tiles with
    a ones column appended (softmax denominator rides along matmul U)
  - scores computed transposed: S.T[k, q] = K.T_h.T @ Q.T_h, one head per
    pass, kt pairs per PSUM tile -> no partition-dim softmax (scores ~N(0,1),
    exp without max-subtraction is safe)
  - exp fused with the 1/8 scale on ACT over 2-bank PSUM tiles
  - U = sum_k [V_h | 1] E: K=128 fp32 accumulation; row 64 = denominators
  - normalize: denominators staged PSUM->SBUF, then reciprocal_approx_fast
    (DVE custom op; correct only with SBUF input on hardware - PSUM input
    returns garbage) -> K=1 ones matmul broadcast -> DVE multiply, deferred a
    few pipeline slots so the DVE chain never blocks the PE
  - out partial: A.T @ Wo per 128-token tile, staged to SBUF, DMA'd out

Measured on trn2 (8 cores, traced): 466us vs 720us for the unpipelined
baseline. Rejected variants: fp8 DoubleRow scores (2x PE rate on QK^T but
6.5e-2 rel err - score quantization sits inside the exp and never averages
out); walrus --enable-ldw-opt=true (incompatible with Tile's split
Ldweights+Matmult form); dummy-LDWEIGHTS p-state padding (net loss).
"""

import heapq
from collections import deque

import numpy as np

B, T, D = 4, 2048, 1024
H, DH = 16, 64
NCORES = 8
HPC = H // NCORES            # heads per core
EB = HPC * DH                # 128-wide embed block per core
TOK = B * T                  # 8192
KT_E = D // 128              # 8 embed k-tiles
NQB = T // 512               # 4 query blocks per batch
KT_T = T // 128              # 16 token k-tiles per batch
NG = KT_T // 2               # 8 score groups (kt pairs) per (qb, head) pass
SCALE = 1.0 / np.sqrt(DH)

_CACHE = {}

import os
OSB_BUFS = int(os.environ.get("K_OSB_BUFS", "2"))
NORMB_DELAY = int(os.environ.get("K_NORMB_DELAY", "3"))
OPROJ_D0 = int(os.environ.get("K_OPROJ_D0", "4"))
OPROJ_STRIDE = int(os.environ.get("K_OPROJ_STRIDE", "1"))
URGENT_CAP = int(os.environ.get("K_URGENT_CAP", "3"))
BG_SLOTS = int(os.environ.get("K_BG_SLOTS", "56"))
PAD_LDW = int(os.environ.get("K_PAD_LDW", "0"))
BG_DELAY = float(os.environ.get("K_BG_DELAY", "0"))


def _build():
    if "nc" in _CACHE:
        return _CACHE["nc"]

    import concourse.bass as bass  # noqa: F401
    from concourse import bacc
    import concourse.mybir as mybir
    import concourse.tile as tile
    from concourse.masks import make_identity

    F32 = mybir.dt.float32
    F16 = mybir.dt.float16
    EXP = mybir.ActivationFunctionType.Exp

    nc = bacc.Bacc("TRN2", target_bir_lowering=False)

    xt_d = nc.dram_tensor("xt", (D, TOK), F16, kind="ExternalInput")
    wq_d = nc.dram_tensor("wq", (D, EB), F16, kind="ExternalInput")
    wk_d = nc.dram_tensor("wk", (D, EB), F16, kind="ExternalInput")
    wv_d = nc.dram_tensor("wv", (D, EB), F16, kind="ExternalInput")
    wo_d = nc.dram_tensor("wo", (EB, D), F16, kind="ExternalInput")
    out_d = nc.dram_tensor("out", (TOK, D), F16, kind="ExternalOutput")

    xt_r = xt_d[:].rearrange("(kt p) t -> p kt t", p=128)
    wq_r = wq_d[:].rearrange("(kt p) e -> p kt e", p=128)
    wk_r = wk_d[:].rearrange("(kt p) e -> p kt e", p=128)
    wv_r = wv_d[:].rearrange("(kt p) e -> p kt e", p=128)

    with tile.TileContext(nc) as tc:
        with (
            tc.tile_pool(name="const", bufs=1) as const,
            tc.tile_pool(name="qk", bufs=2) as qk_pool,
            tc.tile_pool(name="vv", bufs=2) as v_pool,
            tc.tile_pool(name="aa", bufs=2) as a_pool,
            tc.tile_pool(name="xt", bufs=6) as xt_pool,
            tc.tile_pool(name="ee", bufs=3) as e_pool,
            tc.tile_pool(name="vt", bufs=2) as vt_pool,
            tc.tile_pool(name="nr", bufs=2) as nrm_pool,
            tc.tile_pool(name="oo", bufs=OSB_BUFS) as o_sb_pool,
            tc.tile_pool(name="ps", bufs=1, space="PSUM") as ps,
        ):
            # ---- constants / weights ----
            wq_sb = const.tile([128, KT_E, EB], F16)
            wk_sb = const.tile([128, KT_E, EB], F16)
            wv_sb = const.tile([128, KT_E, EB], F16)
            wo_sb = const.tile([128, D], F16)
            nc.sync.dma_start(wq_sb[:], wq_r)
            nc.sync.dma_start(wk_sb[:], wk_r)
            nc.sync.dma_start(wv_sb[:], wv_r)
            nc.sync.dma_start(wo_sb[:], wo_d[:])

            ident0 = const.tile([128, 128], F32)
            make_identity(nc, ident0[:])
            ident = const.tile([128, 128], F16)
            nc.vector.tensor_copy(ident[:], ident0[:])

            onesrow = const.tile([1, 64], F16)
            nc.vector.memset(onesrow[:], 1.0)
            ones_f = const.tile([128, 1], F16)
            nc.vector.memset(ones_f[:], 1.0)

            # ---- shared state ----
            xt_tiles = {}
            deferred_q = {}
            qk_tiles = {}
            v_tiles = {}
            ab_tiles = {}
            vt_tiles = {}
            u_tiles = {}
            rec_tiles = {}

            # ---- scheduler: filler closures popped between S and U ----
            state = {"slot": 0, "pace": 0.0, "credit": 0.0, "seq": 0}
            urgent = []  # heap of (min_slot, seq, fn)
            background = deque()

            def push_urgent(delay, fn):
                heapq.heappush(urgent, (state["slot"] + delay, state["seq"], fn))
                state["seq"] += 1

            def pop_fillers(drain=False):
                n = 0
                while urgent and (drain or (urgent[0][0] <= state["slot"] and n < URGENT_CAP)):
                    heapq.heappop(urgent)[2]()
                    n += 1
                if drain:
                    while background:
                        background.popleft()()
                    return
                state["credit"] += state["pace"]
                while background and state["credit"] >= 1.0:
                    background.popleft()()
                    state["credit"] -= 1.0
                    n += 1
                if n == 0 and PAD_LDW:
                    # No filler this slot: pad the PE with dummy weight loads
                    # so the tensor engine's DVFS never sees an idle window
                    # (idle drops it to the 1.2GHz p-state for the next ~3us).
                    for _ in range(PAD_LDW):
                        nc.tensor.ldweights(ident[:])

            # ---- projection producers (fillers for batch b, run in b-1) ----
            def fetch_xt(b, nb):
                t = xt_pool.tile([128, KT_E, 512], F16, tag="xt", name=f"xt{b}_{nb}")
                c0 = b * T + nb * 512
                nc.sync.dma_start(t[:], xt_r[:, :, c0:c0 + 512])
                xt_tiles[(b, nb)] = t

            def make_proj_qk(b, nb, w_sb, idx):
                def fn():
                    p = ps.tile([128, 512], F32, tag="po", bufs=2,
                                name=f"pp{b}_{nb}_{idx}")
                    xt_t = xt_tiles[(b, nb)]
                    for kt in range(KT_E):
                        nc.tensor.matmul(
                            p[:], w_sb[:, kt, :], xt_t[:, kt, :],
                            start=(kt == 0), stop=(kt == KT_E - 1),
                        )
                    c0 = nb * 512
                    nc.vector.tensor_copy(
                        qk_tiles[b][:, idx, c0:c0 + 512], p[:]
                    )
                return fn

            def make_proj_v(b, nb):
                def fn():
                    p = ps.tile([128, 512], F32, tag="po", bufs=2,
                                name=f"pv{b}_{nb}")
                    xt_t = xt_tiles[(b, nb)]
                    for kt in range(KT_E):
                        nc.tensor.matmul(
                            p[:], wv_sb[:, kt, :], xt_t[:, kt, :],
                            start=(kt == 0), stop=(kt == KT_E - 1),
                        )
                    vt_t = vt_pool.tile([128, 512], F16, tag="vt", name=f"vt{b}_{nb}")
                    nc.vector.tensor_copy(vt_t[:], p[:])
                    vt_tiles[(b, nb)] = vt_t
                return fn

            def make_proj_t(b, nb):
                def fn():
                    vt_t = vt_tiles.pop((b, nb))
                    for i in range(4):
                        tp = ps.tile([128, 128], F16, tag="po", bufs=2,
                                     name=f"tp{b}_{nb}_{i}")
                        with nc.allow_low_precision(reason="fp16 transpose"):
                            nc.tensor.transpose(
                                tp[:], vt_t[:, i * 128:(i + 1) * 128], ident[:]
                            )
                        tokt = nb * 4 + i
                        nc.vector.tensor_copy(
                            v_tiles[b][:, tokt, 0, 0:64], tp[:, 0:64])
                        nc.vector.tensor_copy(
                            v_tiles[b][:, tokt, 1, 0:64], tp[:, 64:128])
                return fn

            def queue_proj(b):
                qk_t = qk_pool.tile([128, 2, T], F16, tag="qk", name=f"qk{b}")
                v_t = v_pool.tile([128, KT_T, HPC, 66], F16, tag="v", name=f"v{b}")
                qk_tiles[b] = qk_t
                v_tiles[b] = v_t
                for kt in range(KT_T):
                    for h in range(HPC):
                        nc.vector.tensor_copy(v_t[:, kt, h, 64:65], ones_f[:])
                for nb in range(NQB):
                    fetch_xt(b, nb)
                for nb in range(NQB):
                    background.append(make_proj_qk(b, nb, wk_sb, 1))
                    if nb < 2:
                        background.append(make_proj_qk(b, nb, wq_sb, 0))
                    background.append(make_proj_v(b, nb))
                    background.append(make_proj_t(b, nb))
                # Q for query blocks 2-3 isn't read until slot ~32/48 of
                # batch b's own attention: defer it there as filler, which
                # also gives the fillerless last batch some PE work.
                deferred_q[b] = [make_proj_qk(b, nb, wq_sb, 0) for nb in (2, 3)]

            # ---- normalize + out-projection (urgent fillers) ----
            def make_norm_a(b, qb, h, u):
                def fn():
                    den32 = nrm_pool.tile([1, 512], F32, tag="den32", bufs=2,
                                          name=f"dn32_{b}_{qb}_{h}")
                    nc.vector.tensor_copy(den32[:], u[64:65, :])
                    rec32 = nrm_pool.tile([1, 512], F32, tag="rec32", bufs=2,
                                          name=f"rc32_{b}_{qb}_{h}")
                    nc.vector.reciprocal_approx_fast(rec32[:], den32[:])
                    rec16 = nrm_pool.tile([1, 512], F16, tag="rec16", bufs=2,
                                          name=f"rc16_{b}_{qb}_{h}")
                    nc.vector.tensor_copy(rec16[:], rec32[:])
                    rec_tiles[(b, qb, h)] = rec16
                return fn

            def make_norm_b(b, qb, h, u):
                def fn():
                    rec16 = rec_tiles.pop((b, qb, h))
                    r_ps = ps.tile([64, 512], F32, tag="po", bufs=2,
                                   name=f"rp{b}_{qb}_{h}")
                    nc.tensor.matmul(
                        r_ps[:], onesrow[:], rec16[:], start=True, stop=True
                    )
                    r_sb = nrm_pool.tile([64, 512], F32, tag="rsb", bufs=2,
                                         name=f"rs{b}_{qb}_{h}")
                    nc.vector.tensor_copy(r_sb[:], r_ps[:])
                    nc.vector.tensor_mul(
                        ab_tiles[b][64 * h:64 * h + 64, qb * 512:(qb + 1) * 512],
                        u[0:64, :], r_sb[:],
                    )
                return fn

            def make_oproj(b, qb, tt):
                def fn():
                    r0 = b * T + qb * 512 + tt * 128
                    c0 = qb * 512 + tt * 128
                    o_sb = o_sb_pool.tile([128, D], F16, tag="o", bufs=OSB_BUFS,
                                          name=f"ob{b}_{qb}_{tt}")
                    for dc in range(2):
                        o_ps = ps.tile([128, 512], F32, tag="po", bufs=2,
                                       name=f"op{b}_{qb}_{tt}_{dc}")
                        nc.tensor.matmul(
                            o_ps[:],
                            ab_tiles[b][:, c0:c0 + 128],
                            wo_sb[:, dc * 512:(dc + 1) * 512],
                            start=True, stop=True,
                        )
                        nc.vector.tensor_copy(
                            o_sb[:, dc * 512:(dc + 1) * 512], o_ps[:]
                        )
                    nc.sync.dma_start(out_d[r0:r0 + 128, :], o_sb[:])
                return fn

            # ---- main pipelined stream ----
            pend = [None]

            def emit_U():
                if pend[0] is None:
                    return
                b, qb, h, g, e = pend[0]
                pend[0] = None
                u = u_tiles[(b, qb, h)]
                for j in range(2):
                    kt = 2 * g + j
                    nc.tensor.matmul(
                        u[:], v_tiles[b][:, kt, h, 0:65],
                        e[:, j * 512:(j + 1) * 512],
                        start=(kt == 0), stop=(kt == KT_T - 1),
                    )
                if g == NG - 1:
                    push_urgent(1, make_norm_a(b, qb, h, u))
                    push_urgent(NORMB_DELAY, make_norm_b(b, qb, h, u))
                    if h == HPC - 1:
                        for tt in range(4):
                            push_urgent(OPROJ_D0 + OPROJ_STRIDE * tt,
                                        make_oproj(b, qb, tt))

            # batch 0 projections run contiguously up front
            queue_proj(0)
            while background:
                background.popleft()()

            for b in range(B):
                for i, fn in enumerate(deferred_q.pop(b, [])):
                    push_urgent(1 + 2 * i, fn)
                if b + 1 < B:
                    queue_proj(b + 1)
                ab = a_pool.tile([128, T], F16, tag="ab", name=f"ab{b}")
                ab_tiles[b] = ab
                state["pace"] = len(background) / float(BG_SLOTS)
                state["credit"] = -BG_DELAY

                for qb in range(NQB):
                    for h in range(HPC):
                        for g in range(NG):
                            s = ps.tile([128, 1024], F32, tag="s", bufs=2,
                                        name=f"s{b}_{qb}_{h}_{g}")
                            for j in range(2):
                                kt = 2 * g + j
                                nc.tensor.matmul(
                                    s[:, j * 512:(j + 1) * 512],
                                    qk_tiles[b][64 * h:64 * h + 64, 1,
                                                kt * 128:(kt + 1) * 128],
                                    qk_tiles[b][64 * h:64 * h + 64, 0,
                                                qb * 512:(qb + 1) * 512],
                                    start=True, stop=True,
                                )
                            e = e_pool.tile([128, 1024], F16, tag="e",
                                            name=f"e{b}_{qb}_{h}_{g}")
                            nc.scalar.activation(e[:], s[:], EXP, scale=SCALE)
                            pop_fillers()
                            emit_U()
                            if g == 0:
                                u_tiles[(b, qb, h)] = ps.tile(
                                    [65, 512], F32, tag="u", bufs=2,
                                    name=f"u{b}_{qb}_{h}",
                                )
                            pend[0] = (b, qb, h, g, e)
                            state["slot"] += 1

            emit_U()
            pop_fillers(drain=True)

    nc.compile()
    _CACHE["nc"] = nc
    return nc


def _run(inputs, trace=False):
    from concourse import bass_utils

    nc = _build()
    x = np.asarray(inputs["x"], dtype=np.float32)
    xt = np.ascontiguousarray(x.reshape(TOK, D).T.astype(np.float16))
    wq = np.asarray(inputs["Wq"], dtype=np.float32)
    wk = np.asarray(inputs["Wk"], dtype=np.float32)
    wv = np.asarray(inputs["Wv"], dtype=np.float32)
    wo = np.asarray(inputs["Wo"], dtype=np.float32)

    in_maps = []
    for c in range(NCORES):
        e0 = c * EB
        in_maps.append({
            "xt": xt,
            "wq": np.ascontiguousarray(wq[e0:e0 + EB, :].T.astype(np.float16)),
            "wk": np.ascontiguousarray(wk[e0:e0 + EB, :].T.astype(np.float16)),
            "wv": np.ascontiguousarray(wv[e0:e0 + EB, :].T.astype(np.float16)),
            "wo": np.ascontiguousarray(wo[:, e0:e0 + EB].T.astype(np.float16)),
        })

    res = bass_utils.run_bass_kernel_spmd(
        nc, in_maps, core_ids=list(range(NCORES)), trace=trace
    )
    acc = res.results[0]["out"].astype(np.float32)
    for c in range(1, NCORES):
        acc = acc + res.results[c]["out"].astype(np.float32)
    out = acc.reshape(B, T, D)
    return out, res


def kernel(x, Wq, Wk, Wv, Wo):
    out, _ = _run({"x": x, "Wq": Wq, "Wk": Wk, "Wv": Wv, "Wo": Wo})
    return out


# revision 24
# speedup vs baseline: 1.2019x; 1.2019x over previous
"""Multi-head self-attention TRN2 kernel, 8-way head-parallel, software-pipelined.

Reference computation (fp32):
    Q = x @ Wq.T; K = x @ Wk.T; V = x @ Wv.T        (split into 16 heads of 64)
    out = softmax(Q K^T / 8) V   per head, concat -> @ Wo.T

Sharding: 2 heads per core (e-block of 128 embed dims). Each core computes
its heads' attention output and a partial out-projection
    out_c = A_c @ Wo[:, e_c].T      (full shape, summed on host)

All matmuls fp16 (1 cyc/row on the PE); PSUM accumulation fp32.

The PE only reaches its 2.4GHz p-state after ~3us of continuous execution and
the in-order engine queue stalls on any not-yet-satisfied dependency, so the
kernel is built as one long software-pipelined instruction stream that keeps
the PE queue saturated:

  - scores run one kt-group ahead of the A.T@V accumulation: PE order is
    S(g+1), <fillers>, U(g), so the ACT exp(g) (the only cross-engine dep)
    completes while S(g+1) streams. Score PSUM tiles are double-buffered
    ([128,1024] x2 = 4 banks).
  - everything that is not scores/U (Q/K/V projections for the NEXT batch,
    the out-projection + softmax normalize of the PREVIOUS query block) is
    queued as filler closures popped between S and U, filling the PE slack
    that ACT's exp pace would otherwise leave.
  - PSUM po-tag tiles (proj, oproj, transpose, norm broadcast) open and meet
    their reader inside a single closure so the 2-buffer rotation can never
    interleave a long-lived accumulation with another alloc (deadlock).

Per-core dataflow (per batch):
  - QT/KT projections produce (128 = 2x64 head dims, T) with the embed
    contraction on partitions (x fed pre-transposed from host)
  - V.T produced the same way, then PE-transposed to (token, dv) tiles with
    a ones column appended (softmax denominator rides along matmul U)
  - scores computed transposed: S.T[k, q] = K.T_h.T @ Q.T_h, one head per
    pass, kt pairs per PSUM tile -> no partition-dim softmax (scores ~N(0,1),
    exp without max-subtraction is safe)
  - exp fused with the 1/8 scale on ACT over 2-bank PSUM tiles
  - U = sum_k [V_h | 1] E: K=128 fp32 accumulation; row 64 = denominators
  - normalize: denominators staged PSUM->SBUF, then reciprocal_approx_fast
    (DVE custom op; correct only with SBUF input on hardware - PSUM input
    returns garbage) -> K=1 ones matmul broadcast -> DVE multiply, deferred a
    few pipeline slots so the DVE chain never blocks the PE
  - out partial: A.T @ Wo per 128-token tile, staged to SBUF, DMA'd out

Measured on trn2 (8 cores, traced): 466us vs 720us for the unpipelined
baseline. Rejected variants: fp8 DoubleRow scores (2x PE rate on QK^T but
6.5e-2 rel err - score quantization sits inside the exp and never averages
out); walrus --enable-ldw-opt=true (incompatible with Tile's split
Ldweights+Matmult form); dummy-LDWEIGHTS p-state padding (net loss).
"""

import heapq
from collections import deque

import numpy as np

B, T, D = 4, 2048, 1024
H, DH = 16, 64
NCORES = 8
HPC = H // NCORES            # heads per core
EB = HPC * DH                # 128-wide embed block per core
TOK = B * T                  # 8192
KT_E = D // 128              # 8 embed k-tiles
NQB = T // 512               # 4 query blocks per batch
KT_T = T // 128              # 16 token k-tiles per batch
NG = KT_T // 2               # 8 score groups (kt pairs) per (qb, head) pass
SCALE = 1.0 / np.sqrt(DH)

_CACHE = {}

import os
OSB_BUFS = int(os.environ.get("K_OSB_BUFS", "2"))
NORMB_DELAY = int(os.environ.get("K_NORMB_DELAY", "3"))
OPROJ_D0 = int(os.environ.get("K_OPROJ_D0", "4"))
OPROJ_STRIDE = int(os.environ.get("K_OPROJ_STRIDE", "1"))
URGENT_CAP = int(os.environ.get("K_URGENT_CAP", "3"))
BG_SLOTS = int(os.environ.get("K_BG_SLOTS", "56"))
PAD_LDW = int(os.environ.get("K_PAD_LDW", "0"))
BG_DELAY = float(os.environ.get("K_BG_DELAY", "0"))


def _build():
    if "nc" in _CACHE:
        return _CACHE["nc"]

    import concourse.bass as bass  # noqa: F401
    from concourse import bacc
    import concourse.mybir as mybir
    import concourse.tile as tile
    from concourse.masks import make_identity

    F32 = mybir.dt.float32
    F16 = mybir.dt.float16
    EXP = mybir.ActivationFunctionType.Exp

    nc = bacc.Bacc("TRN2", target_bir_lowering=False)

    xt_d = nc.dram_tensor("xt", (D, TOK), F16, kind="ExternalInput")
    wq_d = nc.dram_tensor("wq", (D, EB), F16, kind="ExternalInput")
    wk_d = nc.dram_tensor("wk", (D, EB), F16, kind="ExternalInput")
    wv_d = nc.dram_tensor("wv", (D, EB), F16, kind="ExternalInput")
    wo_d = nc.dram_tensor("wo", (EB, D), F16, kind="ExternalInput")
    out_d = nc.dram_tensor("out", (TOK, D), F32, kind="ExternalOutput")

    xt_r = xt_d[:].rearrange("(kt p) t -> p kt t", p=128)
    wq_r = wq_d[:].rearrange("(kt p) e -> p kt e", p=128)
    wk_r = wk_d[:].rearrange("(kt p) e -> p kt e", p=128)
    wv_r = wv_d[:].rearrange("(kt p) e -> p kt e", p=128)

    with tile.TileContext(nc) as tc:
        with (
            tc.tile_pool(name="const", bufs=1) as const,
            tc.tile_pool(name="qk", bufs=2) as qk_pool,
            tc.tile_pool(name="vv", bufs=2) as v_pool,
            tc.tile_pool(name="aa", bufs=2) as a_pool,
            tc.tile_pool(name="xt", bufs=6) as xt_pool,
            tc.tile_pool(name="ee", bufs=3) as e_pool,
            tc.tile_pool(name="vt", bufs=2) as vt_pool,
            tc.tile_pool(name="nr", bufs=2) as nrm_pool,
            tc.tile_pool(name="oo", bufs=OSB_BUFS) as o_sb_pool,
            tc.tile_pool(name="ps", bufs=1, space="PSUM") as ps,
        ):
            # ---- constants / weights ----
            wq_sb = const.tile([128, KT_E, EB], F16)
            wk_sb = const.tile([128, KT_E, EB], F16)
            wv_sb = const.tile([128, KT_E, EB], F16)
            wo_sb = const.tile([128, D], F16)
            nc.sync.dma_start(wq_sb[:], wq_r)
            nc.sync.dma_start(wk_sb[:], wk_r)
            nc.sync.dma_start(wv_sb[:], wv_r)
            nc.sync.dma_start(wo_sb[:], wo_d[:])

            ident0 = const.tile([128, 128], F32)
            make_identity(nc, ident0[:])
            ident = const.tile([128, 128], F16)
            nc.vector.tensor_copy(ident[:], ident0[:])

            onesrow = const.tile([1, 64], F16)
            nc.vector.memset(onesrow[:], 1.0)
            ones_f = const.tile([128, 1], F16)
            nc.vector.memset(ones_f[:], 1.0)

            # ---- shared state ----
            xt_tiles = {}
            deferred_q = {}
            qk_tiles = {}
            v_tiles = {}
            ab_tiles = {}
            vt_tiles = {}
            u_tiles = {}
            rec_tiles = {}

            # ---- scheduler: filler closures popped between S and U ----
            state = {"slot": 0, "pace": 0.0, "credit": 0.0, "seq": 0}
            urgent = []  # heap of (min_slot, seq, fn)
            background = deque()

            def push_urgent(delay, fn):
                heapq.heappush(urgent, (state["slot"] + delay, state["seq"], fn))
                state["seq"] += 1

            def pop_fillers(drain=False):
                n = 0
                while urgent and (drain or (urgent[0][0] <= state["slot"] and n < URGENT_CAP)):
                    heapq.heappop(urgent)[2]()
                    n += 1
                if drain:
                    while background:
                        background.popleft()()
                    return
                state["credit"] += state["pace"]
                while background and state["credit"] >= 1.0:
                    background.popleft()()
                    state["credit"] -= 1.0
                    n += 1
                if n == 0 and PAD_LDW:
                    # No filler this slot: pad the PE with dummy weight loads
                    # so the tensor engine's DVFS never sees an idle window
                    # (idle drops it to the 1.2GHz p-state for the next ~3us).
                    for _ in range(PAD_LDW):
                        nc.tensor.ldweights(ident[:])

            # ---- projection producers (fillers for batch b, run in b-1) ----
            def fetch_xt(b, nb):
                t = xt_pool.tile([128, KT_E, 512], F16, tag="xt", name=f"xt{b}_{nb}")
                c0 = b * T + nb * 512
                nc.sync.dma_start(t[:], xt_r[:, :, c0:c0 + 512])
                xt_tiles[(b, nb)] = t

            def make_proj_qk(b, nb, w_sb, idx):
                def fn():
                    p = ps.tile([128, 512], F32, tag="po", bufs=2,
                                name=f"pp{b}_{nb}_{idx}")
                    xt_t = xt_tiles[(b, nb)]
                    for kt in range(KT_E):
                        nc.tensor.matmul(
                            p[:], w_sb[:, kt, :], xt_t[:, kt, :],
                            start=(kt == 0), stop=(kt == KT_E - 1),
                        )
                    c0 = nb * 512
                    nc.vector.tensor_copy(
                        qk_tiles[b][:, idx, c0:c0 + 512], p[:]
                    )
                return fn

            def make_proj_v(b, nb):
                def fn():
                    p = ps.tile([128, 512], F32, tag="po", bufs=2,
                                name=f"pv{b}_{nb}")
                    xt_t = xt_tiles[(b, nb)]
                    for kt in range(KT_E):
                        nc.tensor.matmul(
                            p[:], wv_sb[:, kt, :], xt_t[:, kt, :],
                            start=(kt == 0), stop=(kt == KT_E - 1),
                        )
                    vt_t = vt_pool.tile([128, 512], F16, tag="vt", name=f"vt{b}_{nb}")
                    nc.vector.tensor_copy(vt_t[:], p[:])
                    vt_tiles[(b, nb)] = vt_t
                return fn

            def make_proj_t(b, nb):
                def fn():
                    vt_t = vt_tiles.pop((b, nb))
                    for i in range(4):
                        tp = ps.tile([128, 128], F16, tag="po", bufs=2,
                                     name=f"tp{b}_{nb}_{i}")
                        with nc.allow_low_precision(reason="fp16 transpose"):
                            nc.tensor.transpose(
                                tp[:], vt_t[:, i * 128:(i + 1) * 128], ident[:]
                            )
                        tokt = nb * 4 + i
                        nc.vector.tensor_copy(
                            v_tiles[b][:, tokt, 0, 0:64], tp[:, 0:64])
                        nc.vector.tensor_copy(
                            v_tiles[b][:, tokt, 1, 0:64], tp[:, 64:128])
                return fn

            def queue_proj(b):
                qk_t = qk_pool.tile([128, 2, T], F16, tag="qk", name=f"qk{b}")
                v_t = v_pool.tile([128, KT_T, HPC, 66], F16, tag="v", name=f"v{b}")
                qk_tiles[b] = qk_t
                v_tiles[b] = v_t
                for kt in range(KT_T):
                    for h in range(HPC):
                        nc.vector.tensor_copy(v_t[:, kt, h, 64:65], ones_f[:])
                for nb in range(NQB):
                    fetch_xt(b, nb)
                for nb in range(NQB):
                    background.append(make_proj_qk(b, nb, wk_sb, 1))
                    if nb < 2:
                        background.append(make_proj_qk(b, nb, wq_sb, 0))
                    background.append(make_proj_v(b, nb))
                    background.append(make_proj_t(b, nb))
                # Q for query blocks 2-3 isn't read until slot ~32/48 of
                # batch b's own attention: defer it there as filler, which
                # also gives the fillerless last batch some PE work.
                deferred_q[b] = [make_proj_qk(b, nb, wq_sb, 0) for nb in (2, 3)]

            # ---- normalize + out-projection (urgent fillers) ----
            def make_norm_a(b, qb, h, u):
                def fn():
                    den32 = nrm_pool.tile([1, 512], F32, tag="den32", bufs=2,
                                          name=f"dn32_{b}_{qb}_{h}")
                    nc.vector.tensor_copy(den32[:], u[64:65, :])
                    rec32 = nrm_pool.tile([1, 512], F32, tag="rec32", bufs=2,
                                          name=f"rc32_{b}_{qb}_{h}")
                    nc.vector.reciprocal_approx_fast(rec32[:], den32[:])
                    rec16 = nrm_pool.tile([1, 512], F16, tag="rec16", bufs=2,
                                          name=f"rc16_{b}_{qb}_{h}")
                    nc.vector.tensor_copy(rec16[:], rec32[:])
                    rec_tiles[(b, qb, h)] = rec16
                return fn

            def make_norm_b(b, qb, h, u):
                def fn():
                    rec16 = rec_tiles.pop((b, qb, h))
                    r_ps = ps.tile([64, 512], F32, tag="po", bufs=2,
                                   name=f"rp{b}_{qb}_{h}")
                    nc.tensor.matmul(
                        r_ps[:], onesrow[:], rec16[:], start=True, stop=True
                    )
                    r_sb = nrm_pool.tile([64, 512], F32, tag="rsb", bufs=2,
                                         name=f"rs{b}_{qb}_{h}")
                    nc.vector.tensor_copy(r_sb[:], r_ps[:])
                    nc.vector.tensor_mul(
                        ab_tiles[b][64 * h:64 * h + 64, qb * 512:(qb + 1) * 512],
                        u[0:64, :], r_sb[:],
                    )
                return fn

            def make_oproj(b, qb, tt):
                def fn():
                    r0 = b * T + qb * 512 + tt * 128
                    c0 = qb * 512 + tt * 128
                    o_sb = o_sb_pool.tile([128, D], F32, tag="o", bufs=OSB_BUFS,
                                          name=f"ob{b}_{qb}_{tt}")
                    for dc in range(2):
                        o_ps = ps.tile([128, 512], F32, tag="po", bufs=2,
                                       name=f"op{b}_{qb}_{tt}_{dc}")
                        nc.tensor.matmul(
                            o_ps[:],
                            ab_tiles[b][:, c0:c0 + 128],
                            wo_sb[:, dc * 512:(dc + 1) * 512],
                            start=True, stop=True,
                        )
                        nc.vector.tensor_copy(
                            o_sb[:, dc * 512:(dc + 1) * 512], o_ps[:]
                        )
                    nc.sync.dma_start(out_d[r0:r0 + 128, :], o_sb[:])
                return fn

            # ---- main pipelined stream ----
            pend = [None]

            def emit_U():
                if pend[0] is None:
                    return
                b, qb, h, g, e = pend[0]
                pend[0] = None
                u = u_tiles[(b, qb, h)]
                for j in range(2):
                    kt = 2 * g + j
                    nc.tensor.matmul(
                        u[:], v_tiles[b][:, kt, h, 0:65],
                        e[:, j * 512:(j + 1) * 512],
                        start=(kt == 0), stop=(kt == KT_T - 1),
                    )
                if g == NG - 1:
                    push_urgent(1, make_norm_a(b, qb, h, u))
                    push_urgent(NORMB_DELAY, make_norm_b(b, qb, h, u))
                    if h == HPC - 1:
                        for tt in range(4):
                            push_urgent(OPROJ_D0 + OPROJ_STRIDE * tt,
                                        make_oproj(b, qb, tt))

            # batch 0 projections run contiguously up front
            queue_proj(0)
            while background:
                background.popleft()()

            for b in range(B):
                for i, fn in enumerate(deferred_q.pop(b, [])):
                    push_urgent(1 + 2 * i, fn)
                if b + 1 < B:
                    queue_proj(b + 1)
                ab = a_pool.tile([128, T], F16, tag="ab", name=f"ab{b}")
                ab_tiles[b] = ab
                state["pace"] = len(background) / float(BG_SLOTS)
                state["credit"] = -BG_DELAY

                for qb in range(NQB):
                    for h in range(HPC):
                        for g in range(NG):
                            s = ps.tile([128, 1024], F32, tag="s", bufs=2,
                                        name=f"s{b}_{qb}_{h}_{g}")
                            for j in range(2):
                                kt = 2 * g + j
                                nc.tensor.matmul(
                                    s[:, j * 512:(j + 1) * 512],
                                    qk_tiles[b][64 * h:64 * h + 64, 1,
                                                kt * 128:(kt + 1) * 128],
                                    qk_tiles[b][64 * h:64 * h + 64, 0,
                                                qb * 512:(qb + 1) * 512],
                                    start=True, stop=True,
                                )
                            e = e_pool.tile([128, 1024], F16, tag="e",
                                            name=f"e{b}_{qb}_{h}_{g}")
                            nc.scalar.activation(e[:], s[:], EXP, scale=SCALE)
                            pop_fillers()
                            emit_U()
                            if g == 0:
                                u_tiles[(b, qb, h)] = ps.tile(
                                    [65, 512], F32, tag="u", bufs=2,
                                    name=f"u{b}_{qb}_{h}",
                                )
                            pend[0] = (b, qb, h, g, e)
                            state["slot"] += 1

            emit_U()
            pop_fillers(drain=True)

    nc.compile()
    _CACHE["nc"] = nc
    return nc


def _run(inputs, trace=False):
    from concourse import bass_utils

    nc = _build()
    x = np.asarray(inputs["x"], dtype=np.float32)
    xt = np.ascontiguousarray(x.reshape(TOK, D).T.astype(np.float16))
    wq = np.asarray(inputs["Wq"], dtype=np.float32)
    wk = np.asarray(inputs["Wk"], dtype=np.float32)
    wv = np.asarray(inputs["Wv"], dtype=np.float32)
    wo = np.asarray(inputs["Wo"], dtype=np.float32)

    in_maps = []
    for c in range(NCORES):
        e0 = c * EB
        in_maps.append({
            "xt": xt,
            "wq": np.ascontiguousarray(wq[e0:e0 + EB, :].T.astype(np.float16)),
            "wk": np.ascontiguousarray(wk[e0:e0 + EB, :].T.astype(np.float16)),
            "wv": np.ascontiguousarray(wv[e0:e0 + EB, :].T.astype(np.float16)),
            "wo": np.ascontiguousarray(wo[:, e0:e0 + EB].T.astype(np.float16)),
        })

    res = bass_utils.run_bass_kernel_spmd(
        nc, in_maps, core_ids=list(range(NCORES)), trace=trace
    )
    acc = res.results[0]["out"]
    for c in range(1, NCORES):
        acc = acc + res.results[c]["out"]
    out = acc.reshape(B, T, D).astype(np.float32)
    return out, res


def kernel(x, Wq, Wk, Wv, Wo):
    out, _ = _run({"x": x, "Wq": Wq, "Wk": Wk, "Wv": Wv, "Wo": Wo})
    return out
